# revision 32
# baseline (speedup 1.0000x reference)
"""Trainium2 Bass kernel for nn_FCNetwork3D (batch-1 dense CNN+MLP).

Network: x[1,2264] -> 6x Conv3d(1,1,3,SAME)+ReLU on the 6x6x6 tail ->
concat -> normalize -> Linear(2264,4096)+tanh -> Linear(4096,4096)+tanh
-> Linear(4096,32) -> scale/shift.

Sharding (8 cores): tensor-parallel on the two wide Linears.
  L0 column-parallel: core k computes h0 block k [512], tanh locally.
  AllGather h0 (1KB/core, bf16) on-device.
  L1 column-parallel: core k computes h1 block k [512], tanh locally.
  L2 row-parallel over h1 blocks: core k computes a partial [1,32];
  host unshard = sum of the 8 partials.
The tiny conv stack runs replicated on every core as 6 matvecs with
host-built conv matrices.

The problem is memory-bound: the weights stream from HBM once per call,
so everything rides in bf16 (half the fp32 bytes; quantization adds
~3e-3 rel err against a 2e-2 budget) as a handful of host-packed
contiguous [128, X] blocks. Bias rows ride as extra K-chunks activated
by a one-hot stationary column.

Schedule (single shared ~330GB/s DMA pipe, served in descriptor-ready
order):
  sync ring pre-streams conv pack -> a0 (3 chunks) -> first 8 K-chunks
  of a1 -> a2, sized so the pipe drains just as h0 finishes; the
  h0-block store (ccin), AllGather and gathered reload (h0g) then cut
  in ahead of the a1 tail, whose descriptor gens are held back by a
  tile_wait_until gate on the scalar ring. The tail streams in ~1us
  chunks so a late h0g is never buried behind more than one chunk.

Compute is in transposed (column) form: the streamed weight tile is
the PE's stationary operand and the x / h0 column the moving one, so
h0/h1 land as [128,4] columns per bank (one PSUM bank per column --
accumulation groups must not interleave within a bank). That makes
tanh per-partition, lets the gather store go out with no extra
transpose, and feeds L2 directly. Warm-up matmuls plus wait-queue
"fillers" before each weight wave keep the PE clock ramped and defer
each wave's dispatch until its tile has landed.
"""

import numpy as np
import ml_dtypes

import concourse.bass as bass
import concourse.mybir as mybir
import concourse.tile as tile
from concourse import bacc
from concourse import bass_utils

F32 = mybir.dt.float32
BF16 = mybir.dt.bfloat16
AF = mybir.ActivationFunctionType
BF_NP = ml_dtypes.bfloat16

NCORES = 8
OBS, ACTD, H, VOX = 2264, 32, 4096, 216
XH = OBS - VOX            # 2048 (x head)
S = H // NCORES           # 512 (per-core block of the hidden dim)
KC0 = 18                  # K-chunks of L0: 16 head + conv128 + conv88/bias
KC1 = 33                  # K-chunks of L1: 32 h0 + bias chunk
A0_COLS = KC0 * S         # 9216
A1_COLS = KC1 * S         # 16896
CONV_COLS = 6 * 432       # per layer: w00[128] | w01[88] | w10[128] | w11[88]
A0_SPLIT = (7, 7, 4)      # K-chunk counts of the three a0 DMA chunks
A1_PRE = 8                # a1 K-chunks pre-streamed on the sync ring
A1_TAIL = (2, 2, 2, 2, 2, 3, 3, 3, 3, 3)  # gated a1 K-chunk groups


def build_nc(reps: int = 1, fake_gather: bool = False):
    """Build the per-core Bass program (identical on all 8 cores; data
    differs via per-core inputs). reps>1 unrolls the whole body for
    steady-state throughput measurement. fake_gather replaces the
    AllGather with a DRAM round-trip + an hrest input (single-core
    TimelineSim oracle)."""
    nc = bacc.Bacc("TRN2", target_bir_lowering=False, debug=False,
                   num_devices=1 if fake_gather else NCORES)

    cons_d = nc.dram_tensor("cons", [128, 26], BF16, kind="ExternalInput")
    cv_d = nc.dram_tensor("cv", [128, CONV_COLS], BF16, kind="ExternalInput")
    a0_d = nc.dram_tensor("a0", [128, A0_COLS], BF16, kind="ExternalInput")
    a1_d = nc.dram_tensor("a1", [128, A1_COLS], BF16, kind="ExternalInput")
    a2_d = nc.dram_tensor("a2", [128, 5 * ACTD], BF16, kind="ExternalInput")
    hrest_d = (nc.dram_tensor("hrest", [H], BF16, kind="ExternalInput")
               if fake_gather else None)
    y_d = nc.dram_tensor("y", [1, ACTD], F32, kind="ExternalOutput")

    with tile.TileContext(nc) as tc:
        with (
            tc.tile_pool(name="cvp", bufs=1) as cvp,
            tc.tile_pool(name="a0p", bufs=3) as a0p,
            tc.tile_pool(name="a1p", bufs=1) as a1p,
            tc.tile_pool(name="a1q", bufs=10) as a1q,  # gate/tail pairs
            tc.tile_pool(name="sp", bufs=6) as sp,
            tc.tile_pool(name="vp", bufs=14) as vp,
            tc.tile_pool(name="ps", bufs=1, space="PSUM") as ps,
            tc.tile_pool(name="psa", bufs=1, space="PSUM") as psa,
            tc.tile_pool(name="psw", bufs=1, space="PSUM") as psw,
            tc.tile_pool(name="psy", bufs=1, space="PSUM") as psy,
            tc.tile_pool(name="dr", bufs=2, space="DRAM") as dr,
        ):
            for _ in range(reps):
                # ===== phase A: DMA issue =====
                # scalar ring: tiny constants (land first), later h0/y.
                cons = sp.tile([128, 26], BF16)
                nc.scalar.dma_start(out=cons[:], in_=cons_d[:, :])
                xt = cons[:, 10:26]
                # cvx1 tail ([1.0, 0...] below the 88 conv rows) preloaded;
                # the last conv layer writes partitions 0:88 later.
                cvx0 = sp.tile([128, 1], BF16)
                cvx1 = sp.tile([128, 1], BF16)
                nc.scalar.dma_start(out=cvx1[88:128, :], in_=cons_d[88:128, 3:4])
                # sync ring: the big weight stream, in consumption order.
                cvw = cvp.tile([128, CONV_COLS], BF16)
                nc.sync.dma_start(out=cvw[:], in_=cv_d[:, :])
                a0t = []
                col = 0
                for n in A0_SPLIT:
                    t = a0p.tile([128, n * S], BF16)
                    nc.sync.dma_start(out=t[:], in_=a0_d[:, col:col + n * S])
                    a0t.append(t)
                    col += n * S
                a1pre = a1p.tile([128, A1_PRE * S], BF16)
                nc.sync.dma_start(out=a1pre[:], in_=a1_d[:, 0:A1_PRE * S])
                a2t = sp.tile([128, 5 * ACTD], BF16)
                nc.sync.dma_start(out=a2t[:], in_=a2_d[:, :])

                # ===== conv stack: 6 serial matvecs =====
                vc0 = cons[:, 0:1]           # v0[0:128]
                vc1 = cons[0:88, 1:2]        # v0[128:216]
                onehot = cons[:, 2:3]        # 1.0 at partition 0

                def fill(lhs, rhs):
                    # wait-queue filler: [1,1]-output matmul on the gate
                    pw = psw.tile([1, 512], F32)
                    nc.tensor.matmul(pw[:, 0:1], lhs, rhs,
                                     start=True, stop=True)

                def warm(rhs):
                    # PE keep-alive: garbage matmul into a scratch PSUM
                    # bank. Covers serial-dependency gaps so the tensor
                    # engine's clock stays ramped for the real stream.
                    n = rhs.shape[-1]
                    pw = psw.tile([1, 512], F32)
                    nc.tensor.matmul(pw[:, 0:n], onehot, rhs,
                                     start=True, stop=True)

                for i in range(6):
                    B = i * 432
                    pm0 = ps.tile([128, 1], F32)
                    pm1 = ps.tile([88, 1], F32)
                    nc.tensor.matmul(pm0[:], cvw[:, B:B + 128], vc0,
                                     start=True, stop=False)
                    nc.tensor.matmul(pm0[:], cvw[0:88, B + 216:B + 344], vc1,
                                     start=False, stop=True)
                    nc.tensor.matmul(pm1[:], cvw[:, B + 128:B + 216], vc0,
                                     start=True, stop=False)
                    nc.tensor.matmul(pm1[:], cvw[0:88, B + 344:B + 432], vc1,
                                     start=False, stop=True)
                    if i == 5:
                        nv0, nv1 = cvx0[:], cvx1[0:88, :]
                    else:
                        nv0t = vp.tile([128, 1], BF16)
                        nv1t = vp.tile([88, 1], BF16)
                        nv0, nv1 = nv0t[:], nv1t[:]
                    nc.scalar.activation(nv0, pm0[:], AF.Relu,
                                         bias=cons[:, 4 + i:5 + i])
                    nc.scalar.activation(nv1, pm1[:], AF.Relu,
                                         bias=cons[0:88, 4 + i:5 + i])
                    vc0, vc1 = nv0, nv1
                    warm(cvw[:, B:B + 432])

                # ===== L0: h0T = tanh(xn @ A0 + b0)^T as columns =====
                # Transposed form: the streamed A0 tile is the PE's
                # stationary operand, the x chunk the moving one, so the
                # output lands as [128, 4] columns -- no PE transpose
                # before the gather store, and tanh is per-partition.
                for c in range(4):
                    fill(xt[:, c:c + 1], a0t[0][:, 0:1])
                # one PSUM bank per output column: accumulation groups
                # must not interleave within a bank
                pac0 = [psa.tile([128, 1], F32, name=f"pac0_{m}")
                        for m in range(S // 128)]
                for c in range(KC0):
                    ti = 0 if c < 7 else (1 if c < 14 else 2)
                    t = a0t[ti]
                    base = (c - (0, 7, 14)[ti]) * S
                    mv = (xt[:, c:c + 1] if c < 16
                          else (cvx0[:] if c == 16 else cvx1[:]))
                    for m in range(S // 128):
                        nc.tensor.matmul(pac0[m][:],
                                         t[:, base + m * 128:base + (m + 1) * 128],
                                         mv, start=(c == 0), stop=(c == KC0 - 1))
                h0T = sp.tile([128, S // 128], BF16)
                for m in range(S // 128):
                    nc.scalar.activation(h0T[:, m:m + 1], pac0[m][:], AF.Tanh)

                # ===== AllGather h0 blocks -> h0g [128, 32] bf16 =====
                # The store permutes h0s so every core's gather block is
                # partition-major: ccin[p*4+j] = h0s[j*128+p]. The reload
                # then runs at 8-byte granularity instead of 2-byte.
                h0g = sp.tile([128, KC1 - 1], BF16)
                ccin = dr.tile([S], BF16)
                nc.scalar.dma_start(
                    out=ccin[:].rearrange("(p j) -> p j", p=128),
                    in_=h0T[:])
                # keep the PE hot across the gather window
                for i in range(12):
                    warm(a0t[1][:, (i % 6) * S:(i % 6) * S + 432])
                if fake_gather:
                    nc.gpsimd.dma_start(
                        out=h0g[:, 0:S // 128],
                        in_=ccin[:].rearrange("(p j) -> p j", p=128))
                    nc.gpsimd.dma_start(
                        out=h0g[:, S // 128:KC1 - 1].rearrange(
                            "p (k j) -> p k j", j=S // 128),
                        in_=hrest_d[S:H].rearrange("(k p j) -> p k j",
                                                   p=128, j=S // 128))
                else:
                    ccout = dr.tile([H], BF16)
                    nc.gpsimd.collective_compute(
                        "AllGather", mybir.AluOpType.bypass,
                        replica_groups=[list(range(NCORES))],
                        ins=[ccin[:].opt()], outs=[ccout[:].opt()])
                    nc.gpsimd.dma_start(
                        out=h0g[:].rearrange("p (k j) -> p k j", j=S // 128),
                        in_=ccout[:].rearrange("(k p j) -> p k j",
                                               p=128, j=S // 128))

                # a1 tail stream, adaptively gated on h0 completion:
                # each tail tile shares a pool buffer (bufs=10) with a
                # tiny "gate" tile written from h0T on the idle vector
                # engine, so the tail's descriptor only becomes ready
                # once h0 is done -- the ccin store and the gather
                # reload (gen'd early on the gpsimd ring) cut ahead of
                # the tail in the shared DMA-engine queue, which serves
                # ready descriptors in ready order.
                for i in range(len(A1_TAIL)):
                    g = a1q.tile([128, 1], BF16, name=f"gate{i}")
                    nc.vector.copy(g[:], h0T[:, 0:1])
                a1tail = []
                col = A1_PRE * S
                for n in A1_TAIL:
                    t = a1q.tile([128, n * S], BF16)
                    nc.scalar.dma_start(out=t[:],
                                        in_=a1_d[:, col:col + n * S])
                    a1tail.append(t)
                    col += n * S

                # ===== L1: h1T = tanh(h0 @ A1 + b1)^T as columns =====
                # fillers before every wave: the wave's matmuls are
                # dispatched only once its tile has landed, so their
                # cost is evaluated with the engine clock already ramped
                for c in range(4):
                    fill(h0g[:, c:c + 1], h0g[:, 0:1])
                pac1 = pac0   # reuse the four banks; L0 is fully read
                c = 0
                for t, n in zip([a1pre] + a1tail, (A1_PRE,) + A1_TAIL):
                    if c > 0:
                        for _ in range(2):
                            fill(h0g[:, 0:1], t[:, 0:1])
                    for j in range(n):
                        mv = (h0g[:, c:c + 1] if c < 32 else onehot)
                        for m in range(S // 128):
                            nc.tensor.matmul(
                                pac1[m][:],
                                t[:, j * S + m * 128:j * S + (m + 1) * 128],
                                mv, start=(c == 0), stop=(c == 32))
                        c += 1
                h1T = sp.tile([128, S // 128], BF16)
                for m in range(S // 128):
                    nc.scalar.activation(h1T[:, m:m + 1], pac1[m][:], AF.Tanh)

                # ===== L2 partial: y_k = h1_blk @ A2_blk + bias'/8 =====
                pyt = psy.tile([1, ACTD], F32)
                py = pyt[:]
                for c in range(S // 128):
                    nc.tensor.matmul(py, h1T[:, c:c + 1],
                                     a2t[:, c * ACTD:(c + 1) * ACTD],
                                     start=(c == 0), stop=False)
                nc.tensor.matmul(py, onehot, a2t[:, 4 * ACTD:5 * ACTD],
                                 start=False, stop=True)
                ys = sp.tile([1, ACTD], F32)
                nc.scalar.copy(ys[:], py)
                nc.scalar.dma_start(out=y_d[:, :], in_=ys[:])

    nc.compile()
    return nc


def _conv_matrix(w: np.ndarray) -> np.ndarray:
    """[216,216] dense matrix of a 3x3x3 SAME cross-correlation on a
    6x6x6 grid: C[o, i] such that y.flat = C @ v.flat."""
    w = np.asarray(w, dtype=np.float32).reshape(3, 3, 3)
    C = np.zeros((VOX, VOX), dtype=np.float32)
    idx = np.arange(6)
    for dz in (-1, 0, 1):
        for dy in (-1, 0, 1):
            for dx in (-1, 0, 1):
                zo, zi = idx[max(0, -dz):6 - max(0, dz)], idx[max(0, dz):6 - max(0, -dz)]
                yo, yi = idx[max(0, -dy):6 - max(0, dy)], idx[max(0, dy):6 - max(0, -dy)]
                xo, xi = idx[max(0, -dx):6 - max(0, dx)], idx[max(0, dx):6 - max(0, -dx)]
                o = (zo[:, None, None] * 36 + yo[None, :, None] * 6 + xo[None, None, :]).ravel()
                i = (zi[:, None, None] * 36 + yi[None, :, None] * 6 + xi[None, None, :]).ravel()
                C[o, i] = w[dz + 1, dy + 1, dx + 1]
    return C


def _chunk_pack(m: np.ndarray, kc: int) -> np.ndarray:
    """[kc*128, X] -> [128, kc*X]: K-chunk c lands in columns c*X:(c+1)*X
    with partition p = row c*128+p."""
    rows, x = m.shape
    assert rows == kc * 128
    return np.ascontiguousarray(
        m.reshape(kc, 128, x).transpose(1, 0, 2).reshape(128, kc * x))


def make_in_maps(inputs: dict) -> list[dict]:
    """Host-side layout prep + sharding: fold normalization into A0,
    out_scale/shift into A2, pre-transpose weights, build conv matrices,
    pack everything into contiguous [128, X] bf16 blocks."""
    f = np.float32
    x = np.asarray(inputs["x"], f)
    W0, b0 = np.asarray(inputs["W0"], f), np.asarray(inputs["b0"], f)
    W1, b1 = np.asarray(inputs["W1"], f), np.asarray(inputs["b1"], f)
    W2, b2 = np.asarray(inputs["W2"], f), np.asarray(inputs["b2"], f)
    in_shift = np.asarray(inputs["in_shift"], f)
    in_scale = np.asarray(inputs["in_scale"], f)
    out_shift = np.asarray(inputs["out_shift"], f)
    out_scale = np.asarray(inputs["out_scale"], f)

    sc = (1.0 / (in_scale.astype(np.float64) + 1e-8)).astype(f)       # [2264]
    A0 = (W0 * sc[None, :]).T.astype(f)                               # [2264, 4096]
    bias0 = (b0 - (in_shift * sc) @ W0.T).astype(f)                   # [4096]
    A1 = W1.T.astype(f)                                               # [4096, 4096]
    A2 = (W2.T * out_scale[None, :]).astype(f)                        # [4096, 32]
    bias2 = ((b2 * out_scale + out_shift) / NCORES).astype(f)         # [32]

    # conv pack: per layer [w00[128,128] | w01[128,88] | w10[88,128] |
    # w11[88,88]] of C^T (stationary = C^T chunk, moving = v chunk).
    cvp = np.zeros((128, CONV_COLS), f)
    for i in range(6):
        CT = _conv_matrix(inputs[f"cw{i}"]).T                         # [216,216]
        B = i * 432
        cvp[:, B:B + 128] = CT[0:128, 0:128]
        cvp[:, B + 128:B + 216] = CT[0:128, 128:216]
        cvp[0:88, B + 216:B + 344] = CT[128:216, 0:128]
        cvp[0:88, B + 344:B + 432] = CT[128:216, 128:216]

    # consts pack [128, 26]: v0 halves, one-hot(p0), cvx1-tail, conv
    # biases, then the x head pre-transposed into K-chunk columns
    cons = np.zeros((128, 26), f)
    v0 = x.ravel()[XH:]
    cons[:, 0] = v0[0:128]
    cons[0:88, 1] = v0[128:216]
    cons[0, 2] = 1.0
    cons[88, 3] = 1.0        # cvx1 tail: 1.0 right below the 88 conv rows
    for i in range(6):
        cons[:, 4 + i] = np.asarray(inputs[f"cb{i}"], f).ravel()[0]
    cons[:, 10:26] = x.ravel()[:XH].reshape(16, 128).T

    in_maps = []
    for k in range(NCORES):
        blk = slice(k * S, (k + 1) * S)
        a0 = np.zeros((KC0 * 128, S), f)
        a0[0:OBS] = A0[:, blk]
        a0[OBS] = bias0[blk]
        a1 = np.zeros((KC1 * 128, S), f)
        a1[0:H] = A1[:, blk]
        a1[H] = b1[blk]
        a2 = np.zeros((5 * 128, ACTD), f)
        a2[0:S] = A2[blk, :]
        a2[S] = bias2
        in_maps.append(dict(
            cons=cons.astype(BF_NP),
            cv=cvp.astype(BF_NP),
            a0=_chunk_pack(a0, KC0).astype(BF_NP),
            a1=_chunk_pack(a1, KC1).astype(BF_NP),
            a2=_chunk_pack(a2, 5).astype(BF_NP),
        ))
    return in_maps


_NC_CACHE: dict = {}


def kernel(**inputs) -> np.ndarray:
    if "nc" not in _NC_CACHE:
        _NC_CACHE["nc"] = build_nc(reps=1)
    nc = _NC_CACHE["nc"]
    in_maps = make_in_maps(inputs)
    res = bass_utils.run_bass_kernel_spmd(nc, in_maps,
                                          core_ids=list(range(NCORES)))
    y = np.sum([res.results[k]["y"] for k in range(NCORES)], axis=0)
    return y.astype(np.float32)
